# revision 68
# baseline (speedup 1.0000x reference)
"""Trainium2 Bass kernel for nn_Attn_48052094107916 (sparse_attention).

Math (per batch b):
  q = x @ Wq.T -> [N, 4, 16];  k = x @ Wk.T -> [N, 4, 16];  v = x @ Wv.T -> [N, 8, 16]
  attn[g,i,j] = <q[i,g,:], k[j,g,:]>
  mw[i,j,g,l] = (masks @ mask_proj)[i,j,g*8+l]
  scores[l,i,j] = sum_g attn[g,i,j] * mw[i,j,g,l]
  out[i,l,:]  = softmax_j(scores[l,i,:]) @ v[:,l,:]

Key restructuring: using mask_proj's rank-3 structure,
  scores[l] = sum_m masks_m (*) w_{m,l},   w_{m,l} = sum_g P[m,g,l] attn_g
and w is computed DIRECTLY on the TensorEngine by scaling q into 24 virtual
heads (qtb), so the per-element work is only 3 masked products + 2 adds +
exp per score plane.

Engine assignment (per key-chunk iteration, all planes [128j, L, RQ]):
  m=0,1 products: DVE scalar_tensor_tensor reading w straight from PSUM
    (fused psum->sbuf crossing + mask multiply, 1 op per plane).
  m=2 product: ACT copies psum->fp16, Pool (gpsimd) TensorTensor multiply,
    batched over a group of 4 chunks.
  the two adds: SWDGE (gpsimd-issued) DMA accumulates p1,p2 into the m0
    product tile -- the adds ride the otherwise-idle DMA engines; Pool only
    pays descriptor-gen (~1us per 4-chunk group).
  exp: ACT, batched over the 4-chunk group.
  q-scaling (24 virtual heads): DVE tensor_scalar (4x packed mode).
Scores stay transposed [j, i]; softmax denominator via an all-ones column in
the PV stationary operand; 17-row PV accumulator block-transposed (DVE) in
the epilogue, normalized, and DMA'd out with the permutation absorbed into
the output access pattern.

Sharding: 8 cores, core r owns query rows [128r, 128r+128) for ALL batches
(sequence parallel).  No collectives.
"""

import os
import sys

import numpy as np

sys.path.insert(0, "/opt/trn_rl_repo")

B, N, C = 8, 1024, 128
G, L, HD = 4, 8, 16
NCORES = 8
RQ = N // NCORES  # query rows per core = 128
NCH = N // 128  # key chunks = 8
GRP = 2  # key chunks per batched group
NGR = NCH // GRP

_cache = {}


def _build():
    import concourse.bacc as bacc
    import concourse.bass as bass
    import concourse.tile as tile
    from concourse import mybir

    f32 = mybir.dt.float32
    bf16 = mybir.dt.bfloat16
    fp16 = mybir.dt.float16
    AF = mybir.ActivationFunctionType
    OP = mybir.AluOpType

    nc = bacc.Bacc("TRN2", target_bir_lowering=False)

    xt_d = nc.dram_tensor("xt", [B, C, N], fp16, kind="ExternalInput")
    xqt_d = nc.dram_tensor("xqt", [B, C, RQ], fp16, kind="ExternalInput")
    mt_d = nc.dram_tensor("maskst", [NCH, 128, 3, 128], fp16, kind="ExternalInput")
    wqt_d = nc.dram_tensor("wqt", [C, 64], fp16, kind="ExternalInput")
    wkt_d = nc.dram_tensor("wkt", [C, 64], fp16, kind="ExternalInput")
    wvt_d = nc.dram_tensor("wvt", [C, C], fp16, kind="ExternalInput")
    pcol_d = nc.dram_tensor("pcol", [64, 3, L], f32, kind="ExternalInput")
    out_d = nc.dram_tensor("out", [B, RQ, C], f32, kind="ExternalOutput")

    with tile.TileContext(nc) as tc, tc.tile_pool(name="singles", bufs=1) as singles, \
            tc.tile_pool(name="xtb", bufs=2) as xtb_pool, \
            tc.tile_pool(name="small", bufs=3) as small, \
            tc.tile_pool(name="cg", bufs=2) as cg_pool, \
            tc.tile_pool(name="scg", bufs=7) as sc_pool, \
            tc.tile_pool(name="p1g", bufs=3) as p1_pool, \
            tc.tile_pool(name="p2g", bufs=4) as p2_pool, \
            tc.tile_pool(name="pbg", bufs=5) as pb_pool, \
            tc.tile_pool(name="epi", bufs=2) as epi, \
            tc.tile_pool(name="w_ps", bufs=3, space="PSUM") as w_ps_pool, \
            tc.tile_pool(name="pv_ps", bufs=1, space="PSUM") as pv_ps:

        # ---------------- resident tensors ----------------
        wqt = singles.tile([C, 64], fp16)
        wkt = singles.tile([C, 64], fp16)
        wvt = singles.tile([C, C], fp16)
        pcol = singles.tile([64, 3, L], f32)
        # only wkt ahead of batch 0's xT load: the first key-chunk products
        # gate the whole ramp, and the HWDGE queue issues strictly in order
        nc.sync.dma_start(out=wkt, in_=wkt_d[:, :])

        xqT = singles.tile([C, B, RQ], fp16)

        masksT = singles.tile([128, NCH, 3, 128], fp16)  # [j, ch, m, i]
        kT = singles.tile([64, B, N], fp16)
        qtb = singles.tile([64, B, 3, L, RQ], fp16)  # P-scaled q, 24 virtual heads
        v17 = singles.tile([128, B, NCH, L, 17], bf16)  # [j, ..., l, d|ones]

        # ones column of v17 (copies below fill [..,0:16])
        nc.gpsimd.memset(v17[:, :, :, :, 16:17], 1.0)

        # persistent epilogue staging: rows 17:32 must read as zero for the
        # 32x32 block transpose; zeroed once here, rows 0:17 rewritten per use
        pv_sb = [singles.tile([32, L, RQ], bf16, name=f"pv_sb{i}")
                 for i in range(2)]
        nc.gpsimd.memset(pv_sb[0], 0.0)
        nc.gpsimd.memset(pv_sb[1], 0.0)

        # ---------------- per-batch projections ----------------
        def proj_load(b):
            xT = xtb_pool.tile([C, N], fp16, tag="xT", name="xT")  # x[b].T
            for h in range(2):
                nc.sync.dma_start(out=xT[:, h * 512:(h + 1) * 512],
                                  in_=xt_d[b, :, h * 512:(h + 1) * 512])
            nc.sync.dma_start(out=xqT[:, b], in_=xqt_d[b])
            return xT

        def proj_a(b, xT=None):
            if xT is None:
                xT = proj_load(b)

            # kT[b] = wkt.T @ xT   [64, N]: one 2-bank psum tile, two
            # matmuls, a single ACT copy (halves the per-op init overhead)
            ps = w_ps_pool.tile([64, 1024], f32, tag="w", name="kps")
            for h in range(2):
                nc.tensor.matmul(ps[:, h * 512:(h + 1) * 512], wkt,
                                 xT[:, h * 512:(h + 1) * 512],
                                 start=True, stop=True)
            nc.scalar.copy(out=kT[:, b, 0:1024], in_=ps)

            # qT[b] = wqt.T @ xqT[b] [64, RQ]
            ps = w_ps_pool.tile([64, 512], f32, tag="w", name="qps")
            nc.tensor.matmul(ps[:, 0:RQ], wqt, xqT[:, b, :], start=True, stop=True)
            qt_sb = small.tile([64, RQ], fp16, tag="qt", name="qt_sb")
            nc.scalar.copy(out=qt_sb, in_=ps[:, 0:RQ])
            return xT, qt_sb

        def proj_b(b, xT, qt_sb, part):
            # P-scaled q copies into qtb (Pool), 6 of 24 per slot -- spread so
            # the burst never jams the Pool queue in front of the dma issues
            for idx in range(6 * part, 6 * (part + 1)):
                m, l = divmod(idx, L)
                nc.vector.tensor_scalar_mul(
                    qtb[:, b, m, l, :], qt_sb, pcol[:, m, l, None])

            # v = x @ Wv.T -> v17 (bf16, strided dst), 2 chunks per slot
            for ch in range(2 * part, 2 * (part + 1)):
                ps = w_ps_pool.tile([128, 128], f32, tag="w", name="vps")
                nc.tensor.matmul(ps, xT[:, ch * 128:(ch + 1) * 128], wvt,
                                 start=True, stop=True)
                nc.vector.tensor_copy(
                    out=v17[:, b, ch, :, 0:16],
                    in_=ps.rearrange("p (l d) -> p l d", l=L),
                )

        # ---------------- score group + PV ----------------
        # products(): w matmuls + psum crossings + mask products for one
        # 2-chunk group.  finish(): the two accumulating DMAs, exp, and PV.
        # finish(g) is emitted AFTER products(g+1) so no compute engine's
        # in-order stream has an op that waits across the group's DMA chain
        # in front of the next group's work.
        def products(b, g):
            ch0 = g * GRP
            sc_g = sc_pool.tile([128, GRP, L, RQ], fp16, tag="sc")
            p1_g = p1_pool.tile([128, GRP, L, RQ], fp16, tag="p1")
            p2_g = p2_pool.tile([128, GRP, L, RQ], fp16, tag="p2")
            cg = cg_pool.tile([128, GRP, L, RQ], fp16, tag="cg")

            def mk(c, m):
                return masksT[:, ch0 + c, m, None, :].to_broadcast((128, L, RQ))

            def wmm(c, m):
                wp = w_ps_pool.tile([128, L, RQ], f32, tag="w", name="wp")
                wpf = wp.rearrange("p l i -> p (l i)")
                qf = qtb[:, b, m].rearrange("p l i -> p (l i)")
                kch = kT[:, b, (ch0 + c) * 128:(ch0 + c + 1) * 128]
                for h in range(2):  # psum bank limit: <=512 f32 per matmul
                    nc.tensor.matmul(
                        wpf[:, h * 512:(h + 1) * 512], kch,
                        qf[:, h * 512:(h + 1) * 512],
                        start=True, stop=True,
                    )
                return wp

            c1 = cg_pool.tile([128, GRP, L, RQ], fp16, tag="c1")
            for c in range(GRP):
                # m=0: fused psum crossing + mask product on DVE
                wp = wmm(c, 0)
                nc.vector.scalar_tensor_tensor(
                    out=sc_g[:, c], in0=wp, scalar=1.0, in1=mk(c, 0),
                    op0=OP.mult, op1=OP.mult)
                # m=1,2: ACT copies psum->fp16; products on DVE (2x packed)
                wp = wmm(c, 1)
                nc.scalar.copy(out=c1[:, c], in_=wp)
                nc.vector.tensor_tensor(
                    out=p1_g[:, c], in0=c1[:, c], in1=mk(c, 1), op=OP.mult)
                wp = wmm(c, 2)
                nc.scalar.copy(out=cg[:, c], in_=wp)
                nc.vector.tensor_tensor(
                    out=p2_g[:, c], in0=cg[:, c], in1=mk(c, 2), op=OP.mult)
            return sc_g, p1_g, p2_g, cg

        # the adds are SWDGE accumulating DMAs (they ride the DMA engines);
        # 4 KiB per partition per transfer (8 KiB single DMAs fault).  The two
        # DMAs and the exp are emitted one group-slot apart so each lands at
        # its engine queue head with its semaphore already satisfied.
        def add_dma(sc_g, src_g):
            nc.gpsimd.dma_start(
                out=sc_g.rearrange("p c l i -> p (c l i)"),
                in_=src_g.rearrange("p c l i -> p (c l i)"),
                accum_op=OP.add)

        def exp_pv(b, g, pv, sc_g):
            ch0 = g * GRP
            pb_g = pb_pool.tile([128, GRP, L, RQ], bf16, tag="pb")
            nc.scalar.activation(out=pb_g, in_=sc_g, func=AF.Exp)
            for c in range(GRP):
                ch = ch0 + c
                for l in range(L):
                    # start=True clears has_written for the WHOLE psum bank:
                    # only the first matmul touching each bank may set it
                    # (pv spans 2 banks: l 0-3 and l 4-7).
                    nc.tensor.matmul(
                        pv[:, l, :],
                        v17[:, b, ch, l, :],
                        pb_g[:, c, l, :],
                        start=(ch == 0 and l % 4 == 0), stop=(ch == NCH - 1),
                        skip_group_check=True,
                    )

        def epilogue(b, pv):
            # 32x32 block-transpose of pv, normalize, store.
            # tr[i%32, l, i//32, c] = pv[c, l, i]; row c=16 is the denom.
            sb = pv_sb[b % 2]
            nc.scalar.copy(out=sb[0:17], in_=pv)
            tr = epi.tile([32, L, 4, 32], bf16, tag="pvtr")
            nc.vector.transpose(
                out=tr.rearrange("p l k r -> p (l k r)"),
                in_=sb.rearrange("p l i -> p (l i)"),
            )
            denr = epi.tile([32, L, 4], f32, tag="denr")
            nc.vector.reciprocal(out=denr, in_=tr[:, :, :, 16])
            ob = epi.tile([32, L, 4, 16], f32, tag="ob")
            nc.vector.tensor_tensor(
                out=ob,
                in0=tr[:, :, :, 0:16],
                in1=denr[:, :, :, None].to_broadcast((32, L, 4, 16)),
                op=OP.mult,
            )
            # out[b, kb*32+r, l*16+d] <- ob[r, l, kb, d]
            ob_dst = bass.AP(
                tensor=out_d, offset=b * RQ * C,
                ap=[[C, 32], [16, L], [32 * C, 4], [1, 16]],
            )
            nc.sync.dma_start(out=ob_dst, in_=ob)

        # ---------------- schedule ----------------
        xTpre = proj_load(0)
        nc.sync.dma_start(out=wqt, in_=wqt_d[:, :])
        nc.sync.dma_start(out=wvt, in_=wvt_d[:, :])
        nc.sync.dma_start(out=pcol, in_=pcol_d[:, :, :])
        xT0, qt0 = proj_a(0, xTpre)
        for part in range(4):
            proj_b(0, xT0, qt0, part)
        for ch in range(NCH):
            nc.sync.dma_start(out=masksT[:, ch], in_=mt_d[ch])

        carry = None
        slots = []  # (b, g, pv, sc, p1, p2), staged 3 slots deep

        def stage1(slot):
            _, _, _, sc_g, p1_g, _, _ = slot
            add_dma(sc_g, p1_g)

        def stage2(slot):
            _, _, _, sc_g, _, p2_g, _ = slot
            add_dma(sc_g, p2_g)

        def stage3(slot):
            sb_, sg_, spv, sc_g, _, _, _ = slot
            exp_pv(sb_, sg_, spv, sc_g)
            if sg_ == NGR - 1:
                epilogue(sb_, spv)

        for b in range(B):
            pv_cur = pv_ps.tile([17, L, RQ], f32, tag="pv", name="pv_cur")
            for g in range(NGR):
                tiles = products(b, g)
                slots.append((b, g, pv_cur, *tiles))
                if len(slots) >= 2:
                    stage1(slots[-2])
                if len(slots) >= 3:
                    stage2(slots[-3])
                if len(slots) >= 5:
                    stage3(slots.pop(0))
                # interleave next batch's projections with this batch's groups
                if b + 1 < B:
                    if g == 0:
                        carry = proj_a(b + 1)
                        proj_b(b + 1, *carry, 0)
                    else:
                        proj_b(b + 1, *carry, g)
        # drain: engine adds instead of accum-DMAs -- at the tail the
        # engines are idling, so the shorter DVE latency wins
        n = len(slots)
        for i, slot in enumerate(slots):
            _, _, _, sc_g, p1_g, p2_g, _ = slot
            if n - i == 1:
                nc.vector.tensor_tensor(out=sc_g, in0=sc_g, in1=p1_g,
                                        op=OP.add)
            if n - i <= 2:
                nc.vector.tensor_tensor(out=sc_g, in0=sc_g, in1=p2_g,
                                        op=OP.add)
        for slot in slots:
            stage3(slot)
        assert n <= 4

    nc.compile()
    return nc


def _get_graph():
    if "nc" not in _cache:
        _cache["nc"] = _build()
    return _cache["nc"]


def kernel(x, masks, Wq, Wk, Wv, mask_proj):
    from concourse import bass_utils

    x = np.asarray(x, dtype=np.float32)
    masks = np.asarray(masks, dtype=np.float32)
    Wq = np.asarray(Wq, dtype=np.float32)
    Wk = np.asarray(Wk, dtype=np.float32)
    Wv = np.asarray(Wv, dtype=np.float32)
    mask_proj = np.asarray(mask_proj, dtype=np.float32)

    f16 = np.float16
    xt = np.ascontiguousarray(x.transpose(0, 2, 1)).astype(f16)  # [B, C, N]
    wqt = np.ascontiguousarray(Wq.T).astype(f16)
    wkt = np.ascontiguousarray(Wk.T).astype(f16)
    wvt = np.ascontiguousarray(Wv.T).astype(f16)
    # pcol[gd, m, l] = mask_proj[m, g(gd)*L + l]
    g_of = (np.arange(64) // HD)
    pcol = np.ascontiguousarray(
        mask_proj[None, :, :].repeat(64, 0)[
            np.arange(64)[:, None, None],
            np.arange(3)[None, :, None],
            (g_of[:, None, None] * L + np.arange(L)[None, None, :])]
    ).astype(np.float32)

    in_maps = []
    for r in range(NCORES):
        sl = slice(r * RQ, (r + 1) * RQ)
        # maskst[ch, j, m, i] = masks[r*128+i, ch*128+j, m]
        msl = masks[sl]  # [i=128, N, 3]
        mt = np.ascontiguousarray(
            msl.reshape(RQ, NCH, 128, 3).transpose(1, 2, 3, 0)).astype(f16)
        in_maps.append({
            "xt": xt,
            "xqt": np.ascontiguousarray(xt[:, :, sl]),
            "maskst": mt,
            "wqt": wqt, "wkt": wkt, "wvt": wvt, "pcol": pcol,
        })

    nc = _get_graph()
    trace = bool(int(os.environ.get("KBENCH_TRACE", "0")))
    try:
        res = bass_utils.run_bass_kernel_spmd(
            nc, in_maps, core_ids=list(range(NCORES)), trace=trace,
        )
    except (ImportError, ModuleNotFoundError):
        res = bass_utils.run_bass_kernel_spmd(
            nc, in_maps, core_ids=list(range(NCORES)), trace=False,
        )
    _cache["last_exec_time_ns"] = getattr(res, "exec_time_ns", None)

    out = np.empty((B, N, C), dtype=np.float32)
    for r in range(NCORES):
        out[:, r * RQ:(r + 1) * RQ, :] = res.results[r]["out"]
    return out
